# revision 104
# baseline (speedup 1.0000x reference)
"""CQAttention (BiDAF context-query attention) Trainium2 kernel.

Shapes: C (32,128,1024), Q (32,128,512), W (32768,1,384) -> out (32,512,1024).
Data-parallel across 8 NeuronCores (4 batches per core, no collectives), per
the batch-axis sharding hint.

Per-batch dataflow (tiles d-major unless noted, 128 partitions):
  psCt  = C^T chunks (PE transpose, consumed directly from PSUM)
  U     = wq + wqc*psCt (DVE mul + Pool add); r = sum_d(wc*psCt) (DVE)
  CtOnes= bf16 [Ct | 1] copied straight from psCt (DVE)
  U^T, Q^T via PE transpose -> DVE copies to SBUF
  S     (c,q) = U^T.T @ Q    -> E = exp(S + r) bf16 (ACT bias=r, accum=rowsum)
  S^T   (q,c) = Q.T @ U^T    -> F = exp(S^T) f32 (ACT), h-major
  G     (q,d+1) = E.T @ [Ct|1] -> colsum in last col; Gn = G[:, :d]/colsum
  rr0   = exp(r)/rowsum (bf16); rrB = transpose-broadcast of rr0 to (d, c)
          via stride-0-lhsT matmuls against a bf16 identity (no DRAM bounce)
  A^T   = (Qt @ F) * rrB (DVE); B^T = (Gn @ F) * rrB (DVE)
  CA    = C * A^T, CB = B^T * C (Pool mid-stream; DVE via CtR=C*rrB for the
          drain batch so Pool never paces the kernel tail)
  out   = [C ; A^T ; CA ; CB]  (C section as DRAM->DRAM copy; A/CA/CB merged
          half-stores on the Pool SWDGE ring; last batch per-section on the
          ACT HWDGE ring so the drain only exposes one small store)

Schedule: emission is software-pipelined across the 4 batches, skewed two
deep -- P1 loads | P2 transposes+U | P3 S->E | P4a F | P4b G+rr+A/B+stores
with P4a delayed one batch and P4b two, so ACT streams
E(0),E(1),F(0),E(2),F(1),E(3),F(2),F(3) essentially gap-free while DVE/PE/
Pool run the adjacent batches' producer and consumer phases underneath.
PSUM: psA (3 banks) rotates the S/F matmul tiles, psB (3 banks) the
transpose/A/B/rr tiles, psG2 (2 banks) the late batches' G accumulators so
the drain-side G phases never queue behind the previous batch's A/B tiles;
engine queues are kept unblocked by emitting work in each engine's true
readiness order.

Engine split: PE matmuls + transposes, ACT exclusively exps (it is the
streaming bottleneck), DVE all PSUM-side elementwise, Pool (GpSimd)
SBUF-only elementwise + mid-stream stores via SWDGE so stores never
head-of-line block input loads on the SP HWDGE queue.
"""

import numpy as np

import concourse.bass as bass
import concourse.bacc as bacc
import concourse.mybir as mybir
from concourse import tile
from concourse.bass_utils import run_bass_kernel_spmd

B, D, CL, QL = 32, 128, 1024, 512
NCORES = 8
BPC = B // NCORES          # batches per core
NC_CHUNK = CL // D         # 8 c-chunks of 128
NQ_CHUNK = QL // D         # 4 q-chunks of 128

F32 = mybir.dt.float32
F32R = mybir.dt.float32r
BF16 = mybir.dt.bfloat16
EXP = mybir.ActivationFunctionType.Exp
ADD = mybir.AluOpType.add

_NC = None
TRACE = False
TRACE_TMPDIR = None
LAST_RESULTS = None


def r32(ap):
    return ap.bitcast(F32R)


def _build():
    nc = bacc.Bacc("TRN2", debug=False, num_devices=NCORES)

    C_d = nc.dram_tensor("C", [BPC, D, CL], F32, kind="ExternalInput").ap()
    Q_d = nc.dram_tensor("Q", [BPC, D, QL], F32, kind="ExternalInput").ap()
    W_d = nc.dram_tensor("W", [BPC, CL, 3 * D], F32, kind="ExternalInput").ap()
    EYE_d = nc.dram_tensor("EYE", [D, D], F32, kind="ExternalInput").ap()
    OUT_d = nc.dram_tensor("OUT", [BPC, 4 * D, CL], F32, kind="ExternalOutput").ap()

    with tile.TileContext(nc) as tc:
        with (
            tc.tile_pool(name="const", bufs=1) as cpool,
            tc.tile_pool(name="work", bufs=2) as pool,
            tc.tile_pool(name="psA", bufs=3, space="PSUM") as psA,
            tc.tile_pool(name="psB", bufs=3, space="PSUM") as psB,
            tc.tile_pool(name="psG2", bufs=2, space="PSUM") as psG2,
        ):
            eye = cpool.tile([D, D], F32)
            eyeb = cpool.tile([D, D], BF16)

            st = [dict() for _ in range(BPC)]
            ctx = (nc, pool, psA, psB, psG2, eye, eyeb, C_d, Q_d, W_d, OUT_d,
                   st, EYE_d)
            # software-pipelined emission, skewed two deep: ACT streams
            # E(0),E(1),F(0),E(2),F(1),E(3),F(2),F(3) with no gaps; each
            # batch's transpose/U chain (P2) gets a full E+F window of slack,
            # and P4b(b) runs against F(b) one E-stream later.
            for b in range(BPC):
                _p1(ctx, b)
                _p2(ctx, b)
                _p3(ctx, b)
                if b >= 1:
                    _p4a(ctx, b - 1)
                if b >= 2:
                    _p4b(ctx, b - 2)
            _p4b(ctx, BPC - 2)
            _p4a(ctx, BPC - 1)
            _p4b(ctx, BPC - 1)
    nc.compile()
    return nc


def _p1(ctx, b):
    """Input loads + the C output section (pure DRAM->DRAM)."""
    nc, pool, psA, psB, psG2, eye, eyeb, C_d, Q_d, W_d, OUT_d, st, EYE_d = ctx
    s = st[b]
    s["C"] = pool.tile([D, CL], F32, tag="Ctile", bufs=4, name=f"Ct{b}")
    s["Q"] = pool.tile([D, QL], F32, tag="Qtile", bufs=3, name=f"Qt{b}")
    s["W"] = pool.tile([D, NC_CHUNK * 3 * D], F32, tag="Wtile", bufs=2, name=f"Wt{b}")
    nc.sync.dma_start(r32(s["C"][:]), r32(C_d[b]))
    if b == 0:
        # eye goes between C and W on the DMA device: C (which the first
        # transposes need) arrives earlier, eye still lands before use
        nc.sync.dma_start(r32(eye[:]), r32(EYE_d[:]))
        nc.vector.tensor_copy(eyeb[:], eye[:])
    nc.sync.dma_start(
        s["W"].rearrange("p (k e) -> p k e", k=NC_CHUNK),
        W_d[b].rearrange("(k p) e -> p k e", p=D),
    )
    nc.sync.dma_start(r32(s["Q"][:]), r32(Q_d[b]))


def _p2(ctx, b):
    """C transposes; U = wq + wqc*Ct, r = sum(wc*Ct), CtOnes; U^T; Q^T."""
    nc, pool, psA, psB, psG2, eye, eyeb, C_d, Q_d, W_d, OUT_d, st, EYE_d = ctx
    s = st[b]
    Ctile, Qtile, Wtile = s["C"], s["Q"], s["W"]
    U = pool.tile([D, CL], F32, tag="U", bufs=2, name=f"U{b}")
    rbias = pool.tile([D, NC_CHUNK], F32, tag="rbias", bufs=3, name=f"rb{b}")
    
    CtOnes = pool.tile([D, NC_CHUNK * (D + 1)], BF16, tag="CtOnes", bufs=3,
                       name=f"co{b}")
    s["U"], s["rbias"], s["CtOnes"] = U, rbias, CtOnes
    co_view = CtOnes.rearrange("p (k d) -> p k d", k=NC_CHUNK)
    nc.gpsimd.memset(co_view[:, :, D:D + 1], 1.0)
    w_view = Wtile.rearrange("p (k e) -> p k e", k=NC_CHUNK)
    u_view = U.rearrange("p (k d) -> p k d", k=NC_CHUNK)
    for g in range(2):  # two groups of 4 chunks per PSUM bank
        ps = psB.tile([D, 4 * D], F32, tag="ps2", padded_shape=[D, QL], name=f"psc{b}{g}")
        rscr = pool.tile([D, 4 * D], F32, tag="rscr", bufs=2, name=f"rs{b}{g}")
        for i in range(4):
            k = 4 * g + i
            nc.tensor.transpose(r32(ps[:, i * D:(i + 1) * D]),
                                r32(Ctile[:, k * D:(k + 1) * D]), r32(eye[:]))
        ps_v = ps.rearrange("p (i d) -> p i d", i=4)
        sl = slice(4 * g, 4 * g + 4)
        nc.vector.tensor_copy(co_view[:, sl, 0:D], ps_v[:])
        nc.vector.tensor_mul(r32(u_view[:, sl]), w_view[:, sl, 2 * D:3 * D], ps_v[:])
        nc.vector.tensor_mul(rscr.rearrange("p (i d) -> p i d", i=4)[:],
                             w_view[:, sl, D:2 * D], ps_v[:])
        uadd = nc.vector if b == 0 else nc.gpsimd
        uadd.tensor_add(r32(u_view[:, sl]), u_view[:, sl], w_view[:, sl, 0:D])
        nc.vector.tensor_reduce(rbias[:, sl],
                                rscr.rearrange("p (i d) -> p i d", i=4)[:],
                                axis=mybir.AxisListType.X, op=ADD)

    UT = pool.tile([D, CL], F32, tag="UT", bufs=3, name=f"UT{b}")
    s["UT"] = UT
    for g in range(2):
        ps = psB.tile([D, 4 * D], F32, tag="ps2", padded_shape=[D, QL], name=f"psu{b}{g}")
        for i in range(4):
            k = 4 * g + i
            nc.tensor.transpose(r32(ps[:, i * D:(i + 1) * D]),
                                r32(U[:, k * D:(k + 1) * D]), r32(eye[:]))
        if b == 0:
            nc.scalar.copy(r32(UT[:, g * 4 * D:(g + 1) * 4 * D]), ps[:])
        else:
            nc.vector.tensor_copy(r32(UT[:, g * 4 * D:(g + 1) * 4 * D]), ps[:])

    Qt = pool.tile([D, QL], F32, tag="Qt", bufs=3, name=f"Qtr{b}")
    s["Qt"] = Qt
    ps = psB.tile([D, 4 * D], F32, tag="ps2", padded_shape=[D, QL], name=f"psq{b}")
    for j in range(NQ_CHUNK):
        nc.tensor.transpose(r32(ps[:, j * D:(j + 1) * D]),
                            r32(Qtile[:, j * D:(j + 1) * D]), r32(eye[:]))
    if b == 0:
        nc.scalar.copy(r32(Qt[:]), ps[:])
    else:
        nc.vector.tensor_copy(r32(Qt[:]), ps[:])


def _p3(ctx, b):
    """S = U^T.T @ Q chunkwise -> E = exp(S + r) bf16 with per-chunk rowsum;
    er = exp(r)."""
    nc, pool, psA, psB, psG2, eye, eyeb, C_d, Q_d, W_d, OUT_d, st, EYE_d = ctx
    s = st[b]
    E = pool.tile([D, NC_CHUNK * QL], BF16, tag="E", bufs=3, name=f"E{b}")
    rowsum = pool.tile([D, NC_CHUNK], F32, tag="rowsum", bufs=3, name=f"rw{b}")
    er = pool.tile([D, NC_CHUNK], F32, tag="er", bufs=3, name=f"er{b}")
    s["E"], s["rowsum"], s["er"] = E, rowsum, er
    for k in range(NC_CHUNK):
        ps = psA.tile([D, QL], F32, tag="ps", name=f"pss{b}{k}")
        nc.tensor.matmul(ps[:], r32(s["UT"][:, k * D:(k + 1) * D]),
                         r32(s["Q"][:]), start=True, stop=True)
        nc.scalar.activation(E[:, k * QL:(k + 1) * QL], ps[:], EXP,
                             bias=s["rbias"][:, k:k + 1],
                             accum_out=rowsum[:, k:k + 1])
    nc.scalar.activation(er[:], s["rbias"][:], EXP)


def _p4a(ctx, b):
    """F = exp(S^T) f32, h-major so each half is ready early."""
    nc, pool, psA, psB, psG2, eye, eyeb, C_d, Q_d, W_d, OUT_d, st, EYE_d = ctx
    s = st[b]
    Ftile = pool.tile([D, NQ_CHUNK * CL], F32, tag="Ftile", bufs=2, name=f"F{b}")
    s["F"] = Ftile
    for h in range(2):
        for j in range(NQ_CHUNK):
            ps = psA.tile([D, QL], F32, tag="ps", name=f"psf{b}{h}{j}")
            nc.tensor.matmul(ps[:], r32(s["Q"][:, j * D:(j + 1) * D]),
                             r32(s["UT"][:, h * QL:(h + 1) * QL]),
                             start=True, stop=True)
            nc.scalar.activation(
                r32(Ftile[:, j * CL + h * QL: j * CL + (h + 1) * QL]), ps[:], EXP)


def _p4b(ctx, b):
    """rr broadcast, G/Gn, A/B matmuls, output scaling + stores (h-split)."""
    nc, pool, psA, psB, psG2, eye, eyeb, C_d, Q_d, W_d, OUT_d, st, EYE_d = ctx
    s = st[b]
    Ctile, E, Ftile, Qt = s["C"], s["E"], s["F"], s["Qt"]

    # C output section: pure DRAM->DRAM, deliberately emitted late so it
    # never competes with input loads for the DMA engines during fill.
    # The last batch's copy is emitted here too (b == BPC-2 iteration) so it
    # doesn't occupy the DMA engines during the drain.
    if b < BPC - 1:
        nc.sync.dma_start(r32(OUT_d[b, 0:D, :]), r32(C_d[b]))
    if b == BPC - 2:
        nc.sync.dma_start(r32(OUT_d[b + 1, 0:D, :]), r32(C_d[b + 1]))

    # rr0 = exp(r)/rowsum (bf16), then transpose-broadcast to rrB (d, c)
    # via stride-0 lhsT matmuls against the bf16 identity; stays in PSUM.
    rs_inv = pool.tile([D, NC_CHUNK], F32, tag="rs_inv", bufs=3, name=f"ri{b}")
    rr0 = pool.tile([D, NC_CHUNK], BF16, tag="rr0", bufs=3, name=f"rr{b}")
    nc.vector.reciprocal(rs_inv[:], s["rowsum"][:])
    nc.vector.tensor_mul(rr0[:], s["er"][:], rs_inv[:])
    rrB_sb = pool.tile([D, CL], F32, tag="rrB", bufs=3, name=f"rrB{b}")
    rrB = [rrB_sb[:, 0:QL], rrB_sb[:, QL:CL]]
    for h in range(2):
        psr = psB.tile([D, QL], F32, tag="ps2", name=f"psr{b}{h}")
        for i in range(4):
            k = 4 * h + i
            nc.tensor.matmul(psr[:, i * D:(i + 1) * D],
                             rr0[:, k:k + 1].broadcast_to([D, D]), eyeb[:],
                             start=True, stop=True)
        nc.vector.tensor_copy(r32(rrB[h]), psr[:])

    # G (q, d+1) = E.T @ [Ct | 1]; Gn = G[:, :d] / colsum
    Gn = pool.tile([D, QL], F32, tag="Gn", bufs=3, name=f"Gn{b}")
    crecip = pool.tile([D, NQ_CHUNK], F32, tag="crecip", bufs=2, name=f"cr{b}")
    CtOnes = s["CtOnes"]
    for j in range(NQ_CHUNK):
        # late batches use the dedicated bank so their G phase never waits
        # behind the previous batch's A/B tiles in the psB rotation
        gpool, gtag = (psG2, "psg") if b >= BPC - 2 else (psB, "ps2")
        psg = gpool.tile([D, D + 1], F32, tag=gtag, padded_shape=[D, QL],
                         name=f"psg{b}{j}")
        for k in range(NC_CHUNK):
            nc.tensor.matmul(psg[:], E[:, k * QL + j * D: k * QL + (j + 1) * D],
                             CtOnes[:, k * (D + 1):(k + 1) * (D + 1)],
                             start=(k == 0), stop=(k == NC_CHUNK - 1))
        nc.vector.reciprocal(crecip[:, j:j + 1], psg[:, D:D + 1])
        nc.vector.tensor_scalar_mul(r32(Gn[:, j * D:(j + 1) * D]), psg[:, 0:D],
                                    crecip[:, j:j + 1])

    # A^T = (Qt @ F) * rrB ; B^T = (Gn @ F) * rrB ; CA, CB; per-half stores
    OutBuf = pool.tile([D, 3 * CL], F32, tag="OutBuf", bufs=3, name=f"OB{b}")
    Asb = OutBuf[:, 0:CL]
    CA = OutBuf[:, CL:2 * CL]
    CB = OutBuf[:, 2 * CL:3 * CL]
    last = b == BPC - 1
    if last:
        # drain batch: CtR = C * rrB lets CA/CB come straight off PSUM with
        # no serial Asb->CA / CBt->CB chains and no Pool in the drain path
        CtR = pool.tile([D, CL], F32, tag="CtR", bufs=1, name=f"CtR{b}")
        nc.vector.tensor_mul(CtR[:, 0:QL], Ctile[:, 0:QL], rrB[0][:])
        nc.vector.tensor_mul(CtR[:, QL:CL], Ctile[:, QL:CL], rrB[1][:])
    for h in range(2):
        hs = slice(h * QL, (h + 1) * QL)
        psa = psB.tile([D, QL], F32, tag="ps2", name=f"psa{b}{h}")
        for j in range(NQ_CHUNK):
            nc.tensor.matmul(psa[:], r32(Qt[:, j * D:(j + 1) * D]),
                             r32(Ftile[:, j * CL + h * QL: j * CL + (h + 1) * QL]),
                             start=(j == 0), stop=(j == NQ_CHUNK - 1))
        nc.vector.tensor_mul(Asb[:, hs], psa[:], rrB[h][:])
        if last:
            nc.vector.tensor_mul(CA[:, hs], psa[:], CtR[:, hs])
        else:
            nc.gpsimd.tensor_mul(CA[:, hs], Ctile[:, hs], Asb[:, hs])
        psb = psB.tile([D, QL], F32, tag="ps2", name=f"psb{b}{h}")
        for j in range(NQ_CHUNK):
            nc.tensor.matmul(psb[:], r32(Gn[:, j * D:(j + 1) * D]),
                             r32(Ftile[:, j * CL + h * QL: j * CL + (h + 1) * QL]),
                             start=(j == 0), stop=(j == NQ_CHUNK - 1))
        if last:
            nc.vector.tensor_mul(CB[:, hs], psb[:], CtR[:, hs])
        else:
            nc.vector.tensor_mul(CB[:, hs], psb[:], rrB[h][:])
            nc.gpsimd.tensor_mul(CB[:, hs], CB[:, hs], Ctile[:, hs])
        # store [A^T | CA | CB] columns of this half (Pool SWDGE ring);
        # the last batch stores in quarters so the drain only exposes one
        nc.gpsimd.dma_start(
            OUT_d[b, D:4 * D, h * QL:(h + 1) * QL].rearrange(
                "(s p) c -> p s c", p=D),
            OutBuf.rearrange("p (s c) -> p s c", s=3)[:, :, h * QL:(h + 1) * QL],
        ) if not last else None
        if last:
            # per-section stores: each fires the moment its section is done
            for sct in range(3):
                nc.scalar.dma_start(
                    OUT_d[b, (1 + sct) * D:(2 + sct) * D, hs],
                    OutBuf[:, sct * CL + h * QL: sct * CL + (h + 1) * QL],
                )


def _get_nc():
    global _NC
    if _NC is None:
        _NC = _build()
    return _NC


def kernel(C, Q, W):
    C = np.ascontiguousarray(np.asarray(C, dtype=np.float32))
    Q = np.ascontiguousarray(np.asarray(Q, dtype=np.float32))
    W = np.ascontiguousarray(np.asarray(W, dtype=np.float32)).reshape(B, CL, 3 * D)
    eye = np.eye(D, dtype=np.float32)
    in_maps = [
        {
            "C": C[i * BPC:(i + 1) * BPC],
            "Q": Q[i * BPC:(i + 1) * BPC],
            "W": W[i * BPC:(i + 1) * BPC],
            "EYE": eye,
        }
        for i in range(NCORES)
    ]
    nc = _get_nc()
    res = run_bass_kernel_spmd(nc, in_maps, core_ids=list(range(NCORES)),
                               trace=TRACE, tmpdir=TRACE_TMPDIR)
    globals()["LAST_RESULTS"] = res
    out = np.concatenate([res.results[i]["OUT"] for i in range(NCORES)], axis=0)
    return out


# revision 105
# speedup vs baseline: 1.0023x; 1.0023x over previous
"""CQAttention (BiDAF context-query attention) Trainium2 kernel.

Shapes: C (32,128,1024), Q (32,128,512), W (32768,1,384) -> out (32,512,1024).
Data-parallel across 8 NeuronCores (4 batches per core, no collectives), per
the batch-axis sharding hint.

Per-batch dataflow (tiles d-major unless noted, 128 partitions):
  psCt  = C^T chunks (PE transpose, consumed directly from PSUM)
  U     = wq + wqc*psCt (DVE mul + Pool add); r = sum_d(wc*psCt) (DVE)
  CtOnes= bf16 [Ct | 1] copied straight from psCt (DVE)
  U^T, Q^T via PE transpose -> DVE copies to SBUF
  S     (c,q) = U^T.T @ Q    -> E = exp(S + r) bf16 (ACT bias=r, accum=rowsum)
  S^T   (q,c) = Q.T @ U^T    -> F = exp(S^T) f32 (ACT), h-major
  G     (q,d+1) = E.T @ [Ct|1] -> colsum in last col; Gn = G[:, :d]/colsum
  rr0   = exp(r)/rowsum (bf16); rrB = transpose-broadcast of rr0 to (d, c)
          via stride-0-lhsT matmuls against a bf16 identity (no DRAM bounce)
  A^T   = (Qt @ F) * rrB (DVE); B^T = (Gn @ F) * rrB (DVE)
  CA    = C * A^T, CB = B^T * C (Pool mid-stream; DVE via CtR=C*rrB for the
          drain batch so Pool never paces the kernel tail)
  out   = [C ; A^T ; CA ; CB]  (C section as DRAM->DRAM copy; A/CA/CB merged
          half-stores on the Pool SWDGE ring; last batch per-section on the
          ACT HWDGE ring so the drain only exposes one small store)

Schedule: emission is software-pipelined across the 4 batches, skewed two
deep -- P1 loads | P2 transposes+U | P3 S->E | P4a F | P4b G+rr+A/B+stores
with P4a delayed one batch and P4b two, so ACT streams
E(0),E(1),F(0),E(2),F(1),E(3),F(2),F(3) essentially gap-free while DVE/PE/
Pool run the adjacent batches' producer and consumer phases underneath.
PSUM: psA (3 banks) rotates the S/F matmul tiles, psB (3 banks) the
transpose/A/B/rr tiles, psG2 (2 banks) the late batches' G accumulators so
the drain-side G phases never queue behind the previous batch's A/B tiles;
engine queues are kept unblocked by emitting work in each engine's true
readiness order.

Engine split: PE matmuls + transposes, ACT exclusively exps (it is the
streaming bottleneck), DVE all PSUM-side elementwise, Pool (GpSimd)
SBUF-only elementwise + mid-stream stores via SWDGE so stores never
head-of-line block input loads on the SP HWDGE queue.
"""

import numpy as np

import concourse.bass as bass
import concourse.bacc as bacc
import concourse.mybir as mybir
from concourse import tile
from concourse.bass_utils import run_bass_kernel_spmd

B, D, CL, QL = 32, 128, 1024, 512
NCORES = 8
BPC = B // NCORES          # batches per core
NC_CHUNK = CL // D         # 8 c-chunks of 128
NQ_CHUNK = QL // D         # 4 q-chunks of 128

F32 = mybir.dt.float32
F32R = mybir.dt.float32r
BF16 = mybir.dt.bfloat16
EXP = mybir.ActivationFunctionType.Exp
ADD = mybir.AluOpType.add

_NC = None
TRACE = False
TRACE_TMPDIR = None
LAST_RESULTS = None


def r32(ap):
    return ap.bitcast(F32R)


def _build():
    nc = bacc.Bacc("TRN2", debug=False, num_devices=NCORES)

    C_d = nc.dram_tensor("C", [BPC, D, CL], F32, kind="ExternalInput").ap()
    Q_d = nc.dram_tensor("Q", [BPC, D, QL], F32, kind="ExternalInput").ap()
    W_d = nc.dram_tensor("W", [BPC, CL, 3 * D], F32, kind="ExternalInput").ap()
    EYE_d = nc.dram_tensor("EYE", [D, D], F32, kind="ExternalInput").ap()
    OUT_d = nc.dram_tensor("OUT", [BPC, 4 * D, CL], F32, kind="ExternalOutput").ap()

    with tile.TileContext(nc) as tc:
        with (
            tc.tile_pool(name="const", bufs=1) as cpool,
            tc.tile_pool(name="work", bufs=2) as pool,
            tc.tile_pool(name="psA", bufs=3, space="PSUM") as psA,
            tc.tile_pool(name="psB", bufs=3, space="PSUM") as psB,
            tc.tile_pool(name="psG2", bufs=2, space="PSUM") as psG2,
        ):
            eye = cpool.tile([D, D], F32)
            eyeb = cpool.tile([D, D], BF16)

            st = [dict() for _ in range(BPC)]
            ctx = (nc, pool, psA, psB, psG2, eye, eyeb, C_d, Q_d, W_d, OUT_d,
                   st, EYE_d)
            # software-pipelined emission, skewed two deep: ACT streams
            # E(0),E(1),F(0),E(2),F(1),E(3),F(2),F(3) with no gaps; each
            # batch's transpose/U chain (P2) gets a full E+F window of slack,
            # and P4b(b) runs against F(b) one E-stream later.
            for b in range(BPC):
                _p1(ctx, b)
                _p2(ctx, b)
                _p3(ctx, b)
                if b >= 1:
                    _p4a(ctx, b - 1)
                if b >= 2:
                    _p4b(ctx, b - 2)
            _p4b(ctx, BPC - 2)
            _p4a(ctx, BPC - 1)
            _p4b(ctx, BPC - 1)
    nc.compile()
    return nc


def _p1(ctx, b):
    """Input loads + the C output section (pure DRAM->DRAM)."""
    nc, pool, psA, psB, psG2, eye, eyeb, C_d, Q_d, W_d, OUT_d, st, EYE_d = ctx
    s = st[b]
    s["C"] = pool.tile([D, CL], F32, tag="Ctile", bufs=4, name=f"Ct{b}")
    s["Q"] = pool.tile([D, QL], F32, tag="Qtile", bufs=3, name=f"Qt{b}")
    s["W"] = pool.tile([D, NC_CHUNK * 3 * D], F32, tag="Wtile", bufs=2, name=f"Wt{b}")
    nc.sync.dma_start(r32(s["C"][:]), r32(C_d[b]))
    if b == 0:
        # eye goes between C and W on the DMA device: C (which the first
        # transposes need) arrives earlier, eye still lands before use
        nc.sync.dma_start(r32(eye[:]), r32(EYE_d[:]))
        nc.vector.tensor_copy(eyeb[:], eye[:])
    nc.sync.dma_start(
        s["W"].rearrange("p (k e) -> p k e", k=NC_CHUNK),
        W_d[b].rearrange("(k p) e -> p k e", p=D),
    )
    nc.sync.dma_start(r32(s["Q"][:]), r32(Q_d[b]))


def _p2(ctx, b):
    """C transposes; U = wq + wqc*Ct, r = sum(wc*Ct), CtOnes; U^T; Q^T."""
    nc, pool, psA, psB, psG2, eye, eyeb, C_d, Q_d, W_d, OUT_d, st, EYE_d = ctx
    s = st[b]
    Ctile, Qtile, Wtile = s["C"], s["Q"], s["W"]
    U = pool.tile([D, CL], F32, tag="U", bufs=2, name=f"U{b}")
    rbias = pool.tile([D, NC_CHUNK], F32, tag="rbias", bufs=3, name=f"rb{b}")
    
    CtOnes = pool.tile([D, NC_CHUNK * (D + 1)], BF16, tag="CtOnes", bufs=3,
                       name=f"co{b}")
    s["U"], s["rbias"], s["CtOnes"] = U, rbias, CtOnes
    co_view = CtOnes.rearrange("p (k d) -> p k d", k=NC_CHUNK)
    nc.gpsimd.memset(co_view[:, :, D:D + 1], 1.0)
    w_view = Wtile.rearrange("p (k e) -> p k e", k=NC_CHUNK)
    u_view = U.rearrange("p (k d) -> p k d", k=NC_CHUNK)
    for g in range(2):  # two groups of 4 chunks per PSUM bank
        ps = psB.tile([D, 4 * D], F32, tag="ps2", padded_shape=[D, QL], name=f"psc{b}{g}")
        rscr = pool.tile([D, 4 * D], F32, tag="rscr", bufs=2, name=f"rs{b}{g}")
        for i in range(4):
            k = 4 * g + i
            nc.tensor.transpose(r32(ps[:, i * D:(i + 1) * D]),
                                r32(Ctile[:, k * D:(k + 1) * D]), r32(eye[:]))
        ps_v = ps.rearrange("p (i d) -> p i d", i=4)
        sl = slice(4 * g, 4 * g + 4)
        nc.vector.tensor_copy(co_view[:, sl, 0:D], ps_v[:])
        nc.vector.tensor_mul(r32(u_view[:, sl]), w_view[:, sl, 2 * D:3 * D], ps_v[:])
        nc.vector.tensor_mul(rscr.rearrange("p (i d) -> p i d", i=4)[:],
                             w_view[:, sl, D:2 * D], ps_v[:])
        uadd = nc.vector if b == 0 else nc.gpsimd
        uadd.tensor_add(r32(u_view[:, sl]), u_view[:, sl], w_view[:, sl, 0:D])
        nc.vector.tensor_reduce(rbias[:, sl],
                                rscr.rearrange("p (i d) -> p i d", i=4)[:],
                                axis=mybir.AxisListType.X, op=ADD)

    UT = pool.tile([D, CL], F32, tag="UT", bufs=3, name=f"UT{b}")
    s["UT"] = UT
    for g in range(2):
        ps = psB.tile([D, 4 * D], F32, tag="ps2", padded_shape=[D, QL], name=f"psu{b}{g}")
        for i in range(4):
            k = 4 * g + i
            nc.tensor.transpose(r32(ps[:, i * D:(i + 1) * D]),
                                r32(U[:, k * D:(k + 1) * D]), r32(eye[:]))
        if b == 0:
            nc.scalar.copy(r32(UT[:, g * 4 * D:(g + 1) * 4 * D]), ps[:])
        else:
            nc.vector.tensor_copy(r32(UT[:, g * 4 * D:(g + 1) * 4 * D]), ps[:])

    Qt = pool.tile([D, QL], F32, tag="Qt", bufs=3, name=f"Qtr{b}")
    s["Qt"] = Qt
    ps = psB.tile([D, 4 * D], F32, tag="ps2", padded_shape=[D, QL], name=f"psq{b}")
    for j in range(NQ_CHUNK):
        nc.tensor.transpose(r32(ps[:, j * D:(j + 1) * D]),
                            r32(Qtile[:, j * D:(j + 1) * D]), r32(eye[:]))
    if b == 0:
        nc.scalar.copy(r32(Qt[:]), ps[:])
    else:
        nc.vector.tensor_copy(r32(Qt[:]), ps[:])


def _p3(ctx, b):
    """S = U^T.T @ Q chunkwise -> E = exp(S + r) bf16 with per-chunk rowsum;
    er = exp(r)."""
    nc, pool, psA, psB, psG2, eye, eyeb, C_d, Q_d, W_d, OUT_d, st, EYE_d = ctx
    s = st[b]
    E = pool.tile([D, NC_CHUNK * QL], BF16, tag="E", bufs=3, name=f"E{b}")
    rowsum = pool.tile([D, NC_CHUNK], F32, tag="rowsum", bufs=3, name=f"rw{b}")
    er = pool.tile([D, NC_CHUNK], F32, tag="er", bufs=3, name=f"er{b}")
    s["E"], s["rowsum"], s["er"] = E, rowsum, er
    # er first: it only needs rbias, so it fills the ACT micro-gap while
    # the first S matmul of this batch is still in flight
    nc.scalar.activation(er[:], s["rbias"][:], EXP)
    for k in range(NC_CHUNK):
        ps = psA.tile([D, QL], F32, tag="ps", name=f"pss{b}{k}")
        nc.tensor.matmul(ps[:], r32(s["UT"][:, k * D:(k + 1) * D]),
                         r32(s["Q"][:]), start=True, stop=True)
        nc.scalar.activation(E[:, k * QL:(k + 1) * QL], ps[:], EXP,
                             bias=s["rbias"][:, k:k + 1],
                             accum_out=rowsum[:, k:k + 1])


def _p4a(ctx, b):
    """F = exp(S^T) f32, h-major so each half is ready early."""
    nc, pool, psA, psB, psG2, eye, eyeb, C_d, Q_d, W_d, OUT_d, st, EYE_d = ctx
    s = st[b]
    Ftile = pool.tile([D, NQ_CHUNK * CL], F32, tag="Ftile", bufs=2, name=f"F{b}")
    s["F"] = Ftile
    for h in range(2):
        for j in range(NQ_CHUNK):
            ps = psA.tile([D, QL], F32, tag="ps", name=f"psf{b}{h}{j}")
            nc.tensor.matmul(ps[:], r32(s["Q"][:, j * D:(j + 1) * D]),
                             r32(s["UT"][:, h * QL:(h + 1) * QL]),
                             start=True, stop=True)
            nc.scalar.activation(
                r32(Ftile[:, j * CL + h * QL: j * CL + (h + 1) * QL]), ps[:], EXP)


def _p4b(ctx, b):
    """rr broadcast, G/Gn, A/B matmuls, output scaling + stores (h-split)."""
    nc, pool, psA, psB, psG2, eye, eyeb, C_d, Q_d, W_d, OUT_d, st, EYE_d = ctx
    s = st[b]
    Ctile, E, Ftile, Qt = s["C"], s["E"], s["F"], s["Qt"]

    # C output section: pure DRAM->DRAM, deliberately emitted late so it
    # never competes with input loads for the DMA engines during fill.
    # The last batch's copy is emitted here too (b == BPC-2 iteration) so it
    # doesn't occupy the DMA engines during the drain.
    if b < BPC - 1:
        nc.sync.dma_start(r32(OUT_d[b, 0:D, :]), r32(C_d[b]))
    if b == BPC - 2:
        nc.sync.dma_start(r32(OUT_d[b + 1, 0:D, :]), r32(C_d[b + 1]))

    # rr0 = exp(r)/rowsum (bf16), then transpose-broadcast to rrB (d, c)
    # via stride-0 lhsT matmuls against the bf16 identity; stays in PSUM.
    rs_inv = pool.tile([D, NC_CHUNK], F32, tag="rs_inv", bufs=3, name=f"ri{b}")
    rr0 = pool.tile([D, NC_CHUNK], BF16, tag="rr0", bufs=3, name=f"rr{b}")
    nc.vector.reciprocal(rs_inv[:], s["rowsum"][:])
    nc.vector.tensor_mul(rr0[:], s["er"][:], rs_inv[:])
    rrB_sb = pool.tile([D, CL], F32, tag="rrB", bufs=3, name=f"rrB{b}")
    rrB = [rrB_sb[:, 0:QL], rrB_sb[:, QL:CL]]
    for h in range(2):
        psr = psB.tile([D, QL], F32, tag="ps2", name=f"psr{b}{h}")
        for i in range(4):
            k = 4 * h + i
            nc.tensor.matmul(psr[:, i * D:(i + 1) * D],
                             rr0[:, k:k + 1].broadcast_to([D, D]), eyeb[:],
                             start=True, stop=True)
        nc.vector.tensor_copy(r32(rrB[h]), psr[:])

    # G (q, d+1) = E.T @ [Ct | 1]; Gn = G[:, :d] / colsum
    Gn = pool.tile([D, QL], F32, tag="Gn", bufs=3, name=f"Gn{b}")
    crecip = pool.tile([D, NQ_CHUNK], F32, tag="crecip", bufs=2, name=f"cr{b}")
    CtOnes = s["CtOnes"]
    for j in range(NQ_CHUNK):
        # late batches use the dedicated bank so their G phase never waits
        # behind the previous batch's A/B tiles in the psB rotation
        gpool, gtag = (psG2, "psg") if b >= BPC - 2 else (psB, "ps2")
        psg = gpool.tile([D, D + 1], F32, tag=gtag, padded_shape=[D, QL],
                         name=f"psg{b}{j}")
        for k in range(NC_CHUNK):
            nc.tensor.matmul(psg[:], E[:, k * QL + j * D: k * QL + (j + 1) * D],
                             CtOnes[:, k * (D + 1):(k + 1) * (D + 1)],
                             start=(k == 0), stop=(k == NC_CHUNK - 1))
        nc.vector.reciprocal(crecip[:, j:j + 1], psg[:, D:D + 1])
        nc.vector.tensor_scalar_mul(r32(Gn[:, j * D:(j + 1) * D]), psg[:, 0:D],
                                    crecip[:, j:j + 1])

    # A^T = (Qt @ F) * rrB ; B^T = (Gn @ F) * rrB ; CA, CB; per-half stores
    OutBuf = pool.tile([D, 3 * CL], F32, tag="OutBuf", bufs=3, name=f"OB{b}")
    Asb = OutBuf[:, 0:CL]
    CA = OutBuf[:, CL:2 * CL]
    CB = OutBuf[:, 2 * CL:3 * CL]
    last = b == BPC - 1
    if last:
        # drain batch: CtR = C * rrB lets CA/CB come straight off PSUM with
        # no serial Asb->CA / CBt->CB chains and no Pool in the drain path
        CtR = pool.tile([D, CL], F32, tag="CtR", bufs=1, name=f"CtR{b}")
        nc.vector.tensor_mul(CtR[:, 0:QL], Ctile[:, 0:QL], rrB[0][:])
        nc.vector.tensor_mul(CtR[:, QL:CL], Ctile[:, QL:CL], rrB[1][:])
    for h in range(2):
        hs = slice(h * QL, (h + 1) * QL)
        psa = psB.tile([D, QL], F32, tag="ps2", name=f"psa{b}{h}")
        for j in range(NQ_CHUNK):
            nc.tensor.matmul(psa[:], r32(Qt[:, j * D:(j + 1) * D]),
                             r32(Ftile[:, j * CL + h * QL: j * CL + (h + 1) * QL]),
                             start=(j == 0), stop=(j == NQ_CHUNK - 1))
        nc.vector.tensor_mul(Asb[:, hs], psa[:], rrB[h][:])
        if last:
            nc.vector.tensor_mul(CA[:, hs], psa[:], CtR[:, hs])
        else:
            nc.gpsimd.tensor_mul(CA[:, hs], Ctile[:, hs], Asb[:, hs])
        psb = psB.tile([D, QL], F32, tag="ps2", name=f"psb{b}{h}")
        for j in range(NQ_CHUNK):
            nc.tensor.matmul(psb[:], r32(Gn[:, j * D:(j + 1) * D]),
                             r32(Ftile[:, j * CL + h * QL: j * CL + (h + 1) * QL]),
                             start=(j == 0), stop=(j == NQ_CHUNK - 1))
        if last:
            nc.vector.tensor_mul(CB[:, hs], psb[:], CtR[:, hs])
        else:
            nc.vector.tensor_mul(CB[:, hs], psb[:], rrB[h][:])
            nc.gpsimd.tensor_mul(CB[:, hs], CB[:, hs], Ctile[:, hs])
        # store [A^T | CA | CB] columns of this half (Pool SWDGE ring);
        # the last batch stores in quarters so the drain only exposes one
        nc.gpsimd.dma_start(
            OUT_d[b, D:4 * D, h * QL:(h + 1) * QL].rearrange(
                "(s p) c -> p s c", p=D),
            OutBuf.rearrange("p (s c) -> p s c", s=3)[:, :, h * QL:(h + 1) * QL],
        ) if not last else None
        if last:
            # per-section stores: each fires the moment its section is done
            for sct in range(3):
                nc.scalar.dma_start(
                    OUT_d[b, (1 + sct) * D:(2 + sct) * D, hs],
                    OutBuf[:, sct * CL + h * QL: sct * CL + (h + 1) * QL],
                )


def _get_nc():
    global _NC
    if _NC is None:
        _NC = _build()
    return _NC


def kernel(C, Q, W):
    C = np.ascontiguousarray(np.asarray(C, dtype=np.float32))
    Q = np.ascontiguousarray(np.asarray(Q, dtype=np.float32))
    W = np.ascontiguousarray(np.asarray(W, dtype=np.float32)).reshape(B, CL, 3 * D)
    eye = np.eye(D, dtype=np.float32)
    in_maps = [
        {
            "C": C[i * BPC:(i + 1) * BPC],
            "Q": Q[i * BPC:(i + 1) * BPC],
            "W": W[i * BPC:(i + 1) * BPC],
            "EYE": eye,
        }
        for i in range(NCORES)
    ]
    nc = _get_nc()
    res = run_bass_kernel_spmd(nc, in_maps, core_ids=list(range(NCORES)),
                               trace=TRACE, tmpdir=TRACE_TMPDIR)
    globals()["LAST_RESULTS"] = res
    out = np.concatenate([res.results[i]["OUT"] for i in range(NCORES)], axis=0)
    return out
